# revision 3
# baseline (speedup 1.0000x reference)
"""Dense mean-field CRF (2-label Potts, gaussian + bilateral pairwise) on 8
Trainium2 NeuronCores.

Math: the bilateral kernel factorizes as S_spatial (separable, sigma=50) o
B_intensity (gaussian gram on f-values). B is numerically rank<=48, so
B ~= P @ P.T (Nystrom, error ~1e-12) and each mean-field message becomes 48
separable 96x96 convolutions instead of an 85M-entry dense matrix:

    msg_i = sum_r P[i,r] * (Sy (x) Sx)(P[:,r] * w)_i ,   w = 10*(2q-1)

The rank dimension r is sharded across the 8 cores (6 each); partial messages
are summed with one AllReduce per mean-field iteration. The gaussian (sigma=3)
term stays a tiny separable conv, replicated on every core; the elementwise
update (logit/sigmoid) is replicated so q stays bitwise identical everywhere.

Weights are +/-10 (not the 0..20 of Q@K) so f32 partial sums random-walk
instead of growing monotonically - accumulation noise stays ~1e-3, far under
the ~0.02 minimum decision margin, which keeps the trajectory glued to the
exact one.
"""
import sys
sys.path.insert(0, '/opt/trn_rl_repo')
import numpy as np

H = W = 96
N = H * W
NCORES = 8
KRANK = 48
KLOC = KRANK // NCORES
NITER = 5
EPS = 1e-8

_CACHE = {}
LAST_RESULTS = None


# ------------------------- host precomputation -------------------------

def _nystrom_P(f64, krank=KRANK):
    """Rank-k factor P [N, k] with exp(-(fi-fj)^2/400) ~= P @ P.T"""
    t = np.linspace(f64.min() - 1.0, f64.max() + 1.0, 256)
    Ktt = np.exp(-(t[:, None] - t[None, :]) ** 2 / 400.0)
    Kft = np.exp(-(f64[:, None] - t[None, :]) ** 2 / 400.0)
    lam, V = np.linalg.eigh(Ktt)
    keep = lam > lam.max() * 1e-14
    R = V[:, keep] / np.sqrt(lam[keep])
    Praw = Kft @ R
    mu, Wv = np.linalg.eigh(Praw.T @ Praw)
    idx = np.argsort(mu)[::-1][:krank]
    return Praw @ Wv[:, idx]          # float64 [N, krank]


def _host_constants(image, mask):
    img64 = np.asarray(image, dtype=np.float64).reshape(H, W)
    m = np.asarray(mask).reshape(-1)
    f64 = img64.reshape(-1)

    P = _nystrom_P(f64)

    idx = np.arange(96, dtype=np.float64)
    d2 = (idx[:, None] - idx[None, :]) ** 2
    S1 = np.exp(-d2 / 5000.0)
    G1 = np.exp(-d2 / 18.0)
    gsum = G1.sum(1)
    gxgy = (gsum[:, None] * gsum[None, :]).reshape(-1)    # pixel order y*96+x

    b = np.where(m == 0, np.log(EPS), -np.log(EPS))
    C = b - 3.0 * gxgy + 13.0
    q0 = 1.0 / (1.0 + np.exp(-b))

    P3 = P.reshape(H, W, KRANK)                            # [y, x, r]
    to32 = lambda a: np.ascontiguousarray(a, dtype=np.float32)
    per_core = []
    for c in range(NCORES):
        sl = P3[:, :, c * KLOC:(c + 1) * KLOC]             # [y, x, kloc]
        PY2 = np.transpose(sl, (0, 2, 1)).reshape(H, KLOC * W)   # [y, r*96+x]
        PX = np.transpose(sl, (1, 2, 0)).reshape(W, KLOC * H)    # [x, r*96+y]
        per_core.append((to32(PY2), to32(PX)))
    shared = {
        "s1": to32(S1), "g1": to32(G1), "i96": to32(np.eye(96)),
        "cx": to32(C.reshape(H, W).T), "q0x": to32(q0.reshape(H, W).T),
    }
    return per_core, shared


# ------------------------- device program -------------------------

def _build():
    import concourse.bacc as bacc
    import concourse.mybir as mybir
    import concourse.tile as tile

    F32 = mybir.dt.float32
    AF = mybir.ActivationFunctionType
    ALU = mybir.AluOpType
    KW = KLOC * 96

    nc = bacc.Bacc("TRN2", target_bir_lowering=False, debug=False,
                   num_devices=NCORES)

    py2_t = nc.dram_tensor("py2", [96, KW], F32, kind="ExternalInput")
    px_t = nc.dram_tensor("px", [96, KW], F32, kind="ExternalInput")
    s1_t = nc.dram_tensor("s1", [96, 96], F32, kind="ExternalInput")
    g1_t = nc.dram_tensor("g1", [96, 96], F32, kind="ExternalInput")
    i96_t = nc.dram_tensor("i96", [96, 96], F32, kind="ExternalInput")
    cx_t = nc.dram_tensor("cx", [96, 96], F32, kind="ExternalInput")
    q0_t = nc.dram_tensor("q0x", [96, 96], F32, kind="ExternalInput")
    out_t = nc.dram_tensor("logit_out", [96, 96], F32, kind="ExternalOutput")

    with tile.TileContext(nc) as tc:
        with (
            tc.tile_pool(name="const", bufs=1) as cpool,
            tc.tile_pool(name="work", bufs=2) as wpool,
            tc.tile_pool(name="psA", bufs=1, space="PSUM") as psA,
            tc.tile_pool(name="psT", bufs=1, space="PSUM") as psT,
            tc.tile_pool(name="psB", bufs=1, space="PSUM") as psB,
            tc.tile_pool(name="psG", bufs=2, space="PSUM") as psG,
            tc.tile_pool(name="dram", bufs=2, space="DRAM") as dpool,
        ):
            sPY = cpool.tile([96, KW], F32, tag="sPY")
            nc.sync.dma_start(sPY[:], py2_t[:])
            sPX = cpool.tile([96, KW], F32, tag="sPX")
            nc.sync.dma_start(sPX[:], px_t[:])
            sS1 = cpool.tile([96, 96], F32, tag="sS1")
            nc.sync.dma_start(sS1[:], s1_t[:])
            sG1 = cpool.tile([96, 96], F32, tag="sG1")
            nc.sync.dma_start(sG1[:], g1_t[:])
            sI = cpool.tile([96, 96], F32, tag="sI")
            nc.sync.dma_start(sI[:], i96_t[:])
            sC = cpool.tile([96, 96], F32, tag="sC")
            nc.sync.dma_start(sC[:], cx_t[:])
            qx = cpool.tile([96, 96], F32, tag="qx")
            nc.sync.dma_start(qx[:], q0_t[:])

            for it in range(NITER):
                # qy = qx^T (PE transpose via identity)
                pq = psG.tile([96, 96], F32, tag="psg")
                nc.tensor.transpose(pq[:], qx[:], sI[:])
                qy = wpool.tile([96, 96], F32, tag="qy")
                nc.vector.tensor_copy(qy[:], pq[:])
                # wy = 20*qy - 10
                wy = wpool.tile([96, 96], F32, tag="wy")
                nc.scalar.activation(wy[:], qy[:], AF.Copy, bias=-10.0, scale=20.0)
                # WP[y, r*96+x] = PY2 * wy (broadcast over r)
                wp = wpool.tile([96, KW], F32, tag="wp")
                for r in range(KLOC):
                    nc.vector.tensor_mul(wp[:, r * 96:(r + 1) * 96],
                                         sPY[:, r * 96:(r + 1) * 96], wy[:])
                # stage A: y-conv for all (r, x) columns
                pa = psA.tile([96, KW], F32, tag="pa")
                for c0 in range(0, KW, 512):
                    c1 = min(c0 + 512, KW)
                    nc.tensor.matmul(pa[:, c0:c1], sS1[:], wp[:, c0:c1],
                                     start=True, stop=True)
                As = wpool.tile([96, KW], F32, tag="As")
                nc.vector.tensor_copy(As[:], pa[:])
                # per-r PE transposes into 128-aligned PSUM slots
                pt = psT.tile([96, KLOC * 128], F32, tag="pt")
                for r in range(KLOC):
                    nc.tensor.transpose(pt[:, r * 128:r * 128 + 96],
                                        As[:, r * 96:(r + 1) * 96], sI[:])
                Ts = wpool.tile([96, KW], F32, tag="Ts")
                nc.vector.tensor_copy(
                    Ts[:].rearrange("p (r y) -> p r y", r=KLOC),
                    pt[:].rearrange("p (r z) -> p r z", r=KLOC)[:, :, 0:96])
                # stage B: x-conv
                pb = psB.tile([96, KW], F32, tag="pb")
                for c0 in range(0, KW, 512):
                    c1 = min(c0 + 512, KW)
                    nc.tensor.matmul(pb[:, c0:c1], sS1[:], Ts[:, c0:c1],
                                     start=True, stop=True)
                # multiply by PX, reduce over r -> partial msg [x, y]
                mm = wpool.tile([96, KW], F32, tag="mm")
                nc.vector.tensor_mul(mm[:], pb[:], sPX[:])
                msg = wpool.tile([96, 96], F32, tag="msg")
                nc.vector.tensor_reduce(
                    msg[:], mm[:].rearrange("p (r y) -> p y r", r=KLOC),
                    axis=mybir.AxisListType.X, op=ALU.add)
                # AllReduce partial messages across the 8 cores
                cin = dpool.tile([96, 96], F32, tag="cin")
                cout = dpool.tile([96, 96], F32, tag="cout")
                nc.sync.dma_start(cin[:], msg[:])
                nc.gpsimd.collective_compute(
                    "AllReduce", ALU.add,
                    replica_groups=[list(range(NCORES))],
                    ins=[cin[:]], outs=[cout[:]])
                msgf = wpool.tile([96, 96], F32, tag="msgf")
                nc.sync.dma_start(msgf[:], cout[:])
                # gaussian sigma=3 term: convgT = G qx G = G @ (G @ qy)^T
                pg1 = psG.tile([96, 96], F32, tag="psg")
                nc.tensor.matmul(pg1[:], sG1[:], qy[:], start=True, stop=True)
                gt1 = wpool.tile([96, 96], F32, tag="gt1")
                nc.vector.tensor_copy(gt1[:], pg1[:])
                pg2 = psG.tile([96, 96], F32, tag="psg")
                nc.tensor.transpose(pg2[:], gt1[:], sI[:])
                gt2 = wpool.tile([96, 96], F32, tag="gt2")
                nc.vector.tensor_copy(gt2[:], pg2[:])
                pg3 = psG.tile([96, 96], F32, tag="psg")
                nc.tensor.matmul(pg3[:], sG1[:], gt2[:], start=True, stop=True)
                # logit = Cx + msgf + 6*convgT - 26*qx
                e1 = wpool.tile([96, 96], F32, tag="e1")
                nc.vector.tensor_add(e1[:], msgf[:], sC[:])
                e2 = wpool.tile([96, 96], F32, tag="e2")
                nc.vector.tensor_scalar_mul(e2[:], pg3[:], 6.0)
                e3 = wpool.tile([96, 96], F32, tag="e3")
                nc.vector.tensor_add(e3[:], e1[:], e2[:])
                e4 = wpool.tile([96, 96], F32, tag="e4")
                nc.vector.tensor_scalar_mul(e4[:], qx[:], 26.0)
                logit = wpool.tile([96, 96], F32, tag="logit")
                nc.vector.tensor_sub(logit[:], e3[:], e4[:])
                if it == NITER - 1:
                    nc.sync.dma_start(out_t[:], logit[:])
                else:
                    nc.scalar.activation(qx[:], logit[:], AF.Sigmoid)

    nc.compile()
    return nc


def _get_nc():
    if "nc" not in _CACHE:
        _CACHE["nc"] = _build()
    return _CACHE["nc"]


# ------------------------- entry point -------------------------

def kernel(image, mask):
    global LAST_RESULTS
    import os
    from concourse.bass_utils import run_bass_kernel_spmd

    per_core, shared = _host_constants(image, mask)
    nc = _get_nc()
    in_maps = []
    for c in range(NCORES):
        m = dict(shared)
        m["py2"], m["px"] = per_core[c]
        in_maps.append(m)
    trace = bool(int(os.environ.get("KERNEL_TRACE", "0")))
    res = run_bass_kernel_spmd(nc, in_maps, core_ids=list(range(NCORES)),
                               trace=trace)
    LAST_RESULTS = res
    logit_xy = res.results[0]["logit_out"]          # [x, y]
    pred = (logit_xy.T < 0).astype(np.float32).reshape(1, 1, H, W)
    return pred


# revision 7
# speedup vs baseline: 1.3787x; 1.3787x over previous
"""Dense mean-field CRF (2-label Potts, gaussian + bilateral pairwise) on 8
Trainium2 NeuronCores.

Math: the bilateral kernel factorizes as S_spatial (separable, sigma=50) o
B_intensity (gaussian gram on f-values). B is numerically rank<=48, so
B ~= P @ P.T (Nystrom, error ~1e-12) and each mean-field message becomes 48
separable 96x96 convolutions instead of an 85M-entry dense matrix:

    msg_i = sum_r P[i,r] * (Sy (x) Sx)(P[:,r] * w)_i ,   w = 10*(2q-1)

The rank dimension r is sharded across the 8 cores (6 each); partial messages
are summed with one AllReduce per mean-field iteration. The gaussian (sigma=3)
term stays a tiny separable conv, replicated on every core; the elementwise
update (logit/sigmoid) is replicated so q stays bitwise identical everywhere.

Weights are +/-10 (not the 0..20 of Q@K) so f32 partial sums random-walk
instead of growing monotonically - accumulation noise stays ~1e-3, far under
the ~0.02 minimum decision margin, which keeps the trajectory glued to the
exact one.
"""
import sys
sys.path.insert(0, '/opt/trn_rl_repo')
import numpy as np

H = W = 96
N = H * W
NCORES = 8
KRANK = 48
KLOC = KRANK // NCORES
NITER = 5
EPS = 1e-8

_CACHE = {}
LAST_RESULTS = None


# ------------------------- host precomputation -------------------------

def _nystrom_P(f64, krank=KRANK):
    """Rank-k factor P [N, k] with exp(-(fi-fj)^2/400) ~= P @ P.T"""
    t = np.linspace(f64.min() - 1.0, f64.max() + 1.0, 256)
    Ktt = np.exp(-(t[:, None] - t[None, :]) ** 2 / 400.0)
    Kft = np.exp(-(f64[:, None] - t[None, :]) ** 2 / 400.0)
    lam, V = np.linalg.eigh(Ktt)
    keep = lam > lam.max() * 1e-14
    R = V[:, keep] / np.sqrt(lam[keep])
    Praw = Kft @ R
    mu, Wv = np.linalg.eigh(Praw.T @ Praw)
    idx = np.argsort(mu)[::-1][:krank]
    return Praw @ Wv[:, idx]          # float64 [N, krank]


def _host_constants(image, mask):
    img64 = np.asarray(image, dtype=np.float64).reshape(H, W)
    m = np.asarray(mask).reshape(-1)
    f64 = img64.reshape(-1)

    P = _nystrom_P(f64)

    idx = np.arange(96, dtype=np.float64)
    d2 = (idx[:, None] - idx[None, :]) ** 2
    S1 = np.exp(-d2 / 5000.0)
    G1 = np.exp(-d2 / 18.0)
    gsum = G1.sum(1)
    gxgy = (gsum[:, None] * gsum[None, :]).reshape(-1)    # pixel order y*96+x

    b = np.where(m == 0, np.log(EPS), -np.log(EPS))
    C = b - 3.0 * gxgy + 13.0
    q0 = 1.0 / (1.0 + np.exp(-b))

    P3 = P.reshape(H, W, KRANK)                            # [y, x, r]
    to32 = lambda a: np.ascontiguousarray(a, dtype=np.float32)
    per_core = []
    for c in range(NCORES):
        sl = P3[:, :, c * KLOC:(c + 1) * KLOC]             # [y, x, kloc]
        PY2 = np.transpose(sl, (0, 2, 1)).reshape(H, KLOC * W)   # [y, r*96+x]
        PX = np.transpose(sl, (1, 2, 0)).reshape(W, KLOC * H)    # [x, r*96+y]
        per_core.append((to32(PY2), to32(PX)))
    shared = {
        "s1": to32(S1), "g1": to32(G1), "i96": to32(np.eye(96)),
        "cx": to32(C.reshape(H, W).T), "q0x": to32(q0.reshape(H, W).T),
    }
    return per_core, shared


# ------------------------- device program -------------------------

def _build():
    import concourse.bacc as bacc
    import concourse.mybir as mybir
    import concourse.tile as tile

    F32 = mybir.dt.float32
    AF = mybir.ActivationFunctionType
    ALU = mybir.AluOpType
    KW = KLOC * 96

    nc = bacc.Bacc("TRN2", target_bir_lowering=False, debug=False,
                   num_devices=NCORES)

    py2_t = nc.dram_tensor("py2", [96, KW], F32, kind="ExternalInput")
    px_t = nc.dram_tensor("px", [96, KW], F32, kind="ExternalInput")
    s1_t = nc.dram_tensor("s1", [96, 96], F32, kind="ExternalInput")
    g1_t = nc.dram_tensor("g1", [96, 96], F32, kind="ExternalInput")
    i96_t = nc.dram_tensor("i96", [96, 96], F32, kind="ExternalInput")
    cx_t = nc.dram_tensor("cx", [96, 96], F32, kind="ExternalInput")
    q0_t = nc.dram_tensor("q0x", [96, 96], F32, kind="ExternalInput")
    out_t = nc.dram_tensor("logit_out", [96, 96], F32, kind="ExternalOutput")

    with tile.TileContext(nc) as tc:
        with (
            tc.tile_pool(name="const", bufs=1) as cpool,
            tc.tile_pool(name="work", bufs=2) as wpool,
            tc.tile_pool(name="psT", bufs=1, space="PSUM") as psT,
            tc.tile_pool(name="psB", bufs=2, space="PSUM") as psB,
            tc.tile_pool(name="psG", bufs=2, space="PSUM") as psG,
            tc.tile_pool(name="dram", bufs=2, space="DRAM") as dpool,
        ):
            # dummy collective issued first: absorbs the cross-core start
            # skew + global-comm bootstrap cost concurrently with the input
            # DMAs and iteration-1 compute, so the first real AllReduce
            # costs the same ~9us as the steady-state ones.
            dml = dpool.tile([8, 4], F32, tag="dml")
            dmo = dpool.tile([8, 4], F32, tag="dmo")
            nc.gpsimd.collective_compute(
                "AllReduce", mybir.AluOpType.add,
                replica_groups=[list(range(NCORES))],
                ins=[dml[:]], outs=[dmo[:]])

            sPY = cpool.tile([96, KW], F32, tag="sPY")
            nc.sync.dma_start(sPY[:], py2_t[:])
            sPX = cpool.tile([96, KW], F32, tag="sPX")
            nc.sync.dma_start(sPX[:], px_t[:])
            sS1 = cpool.tile([96, 96], F32, tag="sS1")
            nc.sync.dma_start(sS1[:], s1_t[:])
            sG1 = cpool.tile([96, 96], F32, tag="sG1")
            nc.sync.dma_start(sG1[:], g1_t[:])
            sI = cpool.tile([96, 96], F32, tag="sI")
            nc.sync.dma_start(sI[:], i96_t[:])
            sC = cpool.tile([96, 96], F32, tag="sC")
            nc.sync.dma_start(sC[:], cx_t[:])
            qx = cpool.tile([96, 96], F32, tag="qx")
            nc.sync.dma_start(qx[:], q0_t[:])

            for it in range(NITER):
                # qy = qx^T (PE transpose via identity)
                pq = psG.tile([96, 96], F32, tag="psg")
                nc.tensor.transpose(pq[:], qx[:], sI[:])
                qy = wpool.tile([96, 96], F32, tag="qy")
                nc.vector.tensor_copy(qy[:], pq[:])
                # wy = 20*qy - 10
                wy = wpool.tile([96, 96], F32, tag="wy")
                nc.scalar.activation(wy[:], qy[:], AF.Copy, bias=-10.0, scale=20.0)
                # WP[y, r*96+x] = PY2 * wy (wy broadcast over r)
                wp = wpool.tile([96, KW], F32, tag="wp")
                nc.vector.tensor_mul(
                    wp[:].rearrange("p (r x) -> p r x", r=KLOC),
                    sPY[:].rearrange("p (r x) -> p r x", r=KLOC),
                    wy[:].unsqueeze(1).broadcast_to([96, KLOC, 96]))
                # stage A, data-stationary: out_r = WP_r.T @ S1 = (Sy WP_r)^T
                # lands already transposed [x, y]; 128-aligned PSUM slots
                pt = psT.tile([96, KLOC * 128], F32, tag="pt")
                for r in range(KLOC):
                    nc.tensor.matmul(pt[:, r * 128:r * 128 + 96],
                                     wp[:, r * 96:(r + 1) * 96], sS1[:],
                                     start=True, stop=True)
                Ts = wpool.tile([96, KW], F32, tag="Ts")
                nc.vector.tensor_copy(
                    Ts[:].rearrange("p (r y) -> p r y", r=KLOC),
                    pt[:].rearrange("p (r z) -> p r z", r=KLOC)[:, :, 0:96])
                # stage B: x-conv
                pb = psB.tile([96, KW], F32, tag="pb")
                for c0 in range(0, KW, 512):
                    c1 = min(c0 + 512, KW)
                    nc.tensor.matmul(pb[:, c0:c1], sS1[:], Ts[:, c0:c1],
                                     start=True, stop=True)
                # multiply by PX, reduce over r -> partial msg [x, y]
                mm = wpool.tile([96, KW], F32, tag="mm")
                nc.vector.tensor_mul(mm[:], pb[:], sPX[:])
                msg = wpool.tile([96, 96], F32, tag="msg")
                nc.vector.tensor_reduce(
                    msg[:], mm[:].rearrange("p (r y) -> p y r", r=KLOC),
                    axis=mybir.AxisListType.X, op=ALU.add)
                # AllReduce partial messages across the 8 cores
                cin = dpool.tile([96, 96], F32, tag="cin")
                cout = dpool.tile([96, 96], F32, tag="cout")
                nc.sync.dma_start(cin[:], msg[:])
                nc.gpsimd.collective_compute(
                    "AllReduce", ALU.add,
                    replica_groups=[list(range(NCORES))],
                    ins=[cin[:]], outs=[cout[:]])
                msgf = wpool.tile([96, 96], F32, tag="msgf")
                nc.sync.dma_start(msgf[:], cout[:])
                # gaussian sigma=3 term: convgT = G qx G = G @ (G @ qy)^T
                pg1 = psG.tile([96, 96], F32, tag="psg")
                nc.tensor.matmul(pg1[:], sG1[:], qy[:], start=True, stop=True)
                gt1 = wpool.tile([96, 96], F32, tag="gt1")
                nc.vector.tensor_copy(gt1[:], pg1[:])
                pg2 = psG.tile([96, 96], F32, tag="psg")
                nc.tensor.transpose(pg2[:], gt1[:], sI[:])
                gt2 = wpool.tile([96, 96], F32, tag="gt2")
                nc.vector.tensor_copy(gt2[:], pg2[:])
                pg3 = psG.tile([96, 96], F32, tag="psg")
                nc.tensor.matmul(pg3[:], sG1[:], gt2[:], start=True, stop=True)
                # logit = Cx + msgf + 6*convgT - 26*qx
                e1 = wpool.tile([96, 96], F32, tag="e1")
                nc.vector.tensor_add(e1[:], msgf[:], sC[:])
                e2 = wpool.tile([96, 96], F32, tag="e2")
                nc.vector.tensor_scalar_mul(e2[:], pg3[:], 6.0)
                e3 = wpool.tile([96, 96], F32, tag="e3")
                nc.vector.tensor_add(e3[:], e1[:], e2[:])
                e4 = wpool.tile([96, 96], F32, tag="e4")
                nc.vector.tensor_scalar_mul(e4[:], qx[:], 26.0)
                logit = wpool.tile([96, 96], F32, tag="logit")
                nc.vector.tensor_sub(logit[:], e3[:], e4[:])
                if it == NITER - 1:
                    nc.sync.dma_start(out_t[:], logit[:])
                else:
                    nc.scalar.activation(qx[:], logit[:], AF.Sigmoid)

    nc.compile()
    return nc


def _get_nc():
    if "nc" not in _CACHE:
        _CACHE["nc"] = _build()
    return _CACHE["nc"]


# ------------------------- entry point -------------------------

def kernel(image, mask):
    global LAST_RESULTS
    import os
    from concourse.bass_utils import run_bass_kernel_spmd

    per_core, shared = _host_constants(image, mask)
    nc = _get_nc()
    in_maps = []
    for c in range(NCORES):
        m = dict(shared)
        m["py2"], m["px"] = per_core[c]
        in_maps.append(m)
    trace = bool(int(os.environ.get("KERNEL_TRACE", "0")))
    kw = {}
    if trace and os.environ.get("KERNEL_TRACE_ALL"):
        kw["trace_cores"] = list(range(NCORES))
        kw["stitch_traces"] = True
    res = run_bass_kernel_spmd(nc, in_maps, core_ids=list(range(NCORES)),
                               trace=trace, **kw)
    LAST_RESULTS = res
    logit_xy = res.results[0]["logit_out"]          # [x, y]
    pred = (logit_xy.T < 0).astype(np.float32).reshape(1, 1, H, W)
    return pred


# revision 9
# speedup vs baseline: 1.4939x; 1.0836x over previous
"""Dense mean-field CRF (2-label Potts, gaussian + bilateral pairwise) on 8
Trainium2 NeuronCores.

Math: the bilateral kernel factorizes as S_spatial (separable, sigma=50) o
B_intensity (gaussian gram on the pixel values). B is numerically rank<=48,
so B ~= P @ P.T (Nystrom over 256 landmark intensities, error ~1e-12) and
each mean-field message becomes 48 separable 96x96 convolutions instead of an
85M-entry dense matrix:

    msg = sum_r P_r o (Sy (x) Sx)(10 P_r o h),   h = 2q - 1 = tanh(logit/2)

In h-space the update is  logit = b + msg + 3*conv_g(h) - 13*h  (the
self-exclusion and rowsum terms collapse into these coefficients), so one
Tanh is the only activation. Signed h keeps f32 partial sums random-walking;
total logit noise ~1e-3 vs a minimum decision margin of ~0.02, so the
trajectory tracks the exact computation and the argmax output is exact.

Distribution: the rank dim is sharded across the 8 cores (6 each) with one
AllGather + local 8-way sum per iteration. Iteration 1 is instead replicated
at full rank on every core, hiding under the first-collective bootstrap
barrier that a dummy collective absorbs concurrently.
"""
import sys
sys.path.insert(0, '/opt/trn_rl_repo')
import numpy as np

H = W = 96
N = H * W
NCORES = 8
KRANK = 48
KLOC = KRANK // NCORES
NITER = 5
EPS = 1e-8

_CACHE = {}
LAST_RESULTS = None


# ------------------------- host precomputation -------------------------

def _nystrom_P(f64, krank=KRANK):
    """Rank-k factor P [N, k] with exp(-(fi-fj)^2/400) ~= P @ P.T"""
    t = np.linspace(f64.min() - 1.0, f64.max() + 1.0, 256)
    Ktt = np.exp(-(t[:, None] - t[None, :]) ** 2 / 400.0)
    Kft = np.exp(-(f64[:, None] - t[None, :]) ** 2 / 400.0)
    lam, V = np.linalg.eigh(Ktt)
    keep = lam > lam.max() * 1e-14
    R = V[:, keep] / np.sqrt(lam[keep])
    Praw = Kft @ R
    mu, Wv = np.linalg.eigh(Praw.T @ Praw)
    idx = np.argsort(mu)[::-1][:krank]
    return Praw @ Wv[:, idx]          # float64 [N, krank]


def _rmajor(P3):
    """[y, x, r] -> [96, r*96 + x] float32"""
    return np.ascontiguousarray(
        np.transpose(P3, (0, 2, 1)).reshape(H, -1), dtype=np.float32)


def _host_constants(image, mask):
    img64 = np.asarray(image, dtype=np.float64).reshape(H, W)
    m = np.asarray(mask).reshape(-1)
    f64 = img64.reshape(-1)

    P = _nystrom_P(f64)
    P3 = P.reshape(H, W, KRANK)
    P310 = 10.0 * P3

    idx = np.arange(96, dtype=np.float64)
    d2 = (idx[:, None] - idx[None, :]) ** 2
    b = np.where(m == 0, np.log(EPS), -np.log(EPS))

    to32 = lambda a: np.ascontiguousarray(a, dtype=np.float32)
    shared = {
        "s1": to32(np.exp(-d2 / 5000.0)),
        "g1": to32(np.exp(-d2 / 18.0)),
        "i96": to32(np.eye(96)),
        "cb": to32(b.reshape(H, W)),
        "h0": to32(np.tanh(b / 2.0).reshape(H, W)),
        "pyf10": _rmajor(P310),
        "pyfraw": _rmajor(P3),
    }
    per_core = []
    for c in range(NCORES):
        rs = slice(c * KLOC, (c + 1) * KLOC)
        per_core.append((_rmajor(P310[:, :, rs]), _rmajor(P3[:, :, rs])))
    return per_core, shared


# ------------------------- device program -------------------------

def _build():
    import concourse.bacc as bacc
    import concourse.mybir as mybir
    import concourse.tile as tile

    F32 = mybir.dt.float32
    AF = mybir.ActivationFunctionType
    ALU = mybir.AluOpType
    KW = KLOC * 96          # 576
    KWF = KRANK * 96        # 4608
    RG = [list(range(NCORES))]

    nc = bacc.Bacc("TRN2", target_bir_lowering=False, debug=False,
                   num_devices=NCORES)

    t_in = {}
    for name, shape in [("py10", [96, KW]), ("pyraw", [96, KW]),
                        ("pyf10", [96, KWF]), ("pyfraw", [96, KWF]),
                        ("s1", [96, 96]), ("g1", [96, 96]), ("i96", [96, 96]),
                        ("cb", [96, 96]), ("h0", [96, 96])]:
        t_in[name] = nc.dram_tensor(name, shape, F32, kind="ExternalInput")
    out_t = nc.dram_tensor("logit_out", [96, 96], F32, kind="ExternalOutput")

    with tile.TileContext(nc) as tc:
        with (
            tc.tile_pool(name="const", bufs=1) as cpool,
            tc.tile_pool(name="work", bufs=2) as wpool,
            tc.tile_pool(name="psT", bufs=1, space="PSUM") as psT,
            tc.tile_pool(name="psB", bufs=2, space="PSUM") as psB,
            tc.tile_pool(name="psG", bufs=2, space="PSUM") as psG,
            tc.tile_pool(name="dram", bufs=2, space="DRAM") as dpool,
        ):
            # dummy collective first: absorbs cross-core start skew + comm
            # bootstrap concurrently with input DMAs and iteration 1.
            dml = dpool.tile([8, 4], F32, tag="dml")
            dmo = dpool.tile([64, 4], F32, tag="dmo")
            nc.gpsimd.collective_compute(
                "AllGather", ALU.bypass, replica_groups=RG,
                ins=[dml[:]], outs=[dmo[:]])

            sb = {}
            for name in t_in:
                sb[name] = cpool.tile(list(t_in[name].shape), F32, tag=name,
                                      name=f"sb_{name}")
                nc.sync.dma_start(sb[name][:], t_in[name][:])
            hy = cpool.tile([96, 96], F32, tag="hy")
            nc.sync.dma_start(hy[:], t_in["h0"][:])

            def bilateral_partial(p10, praw, kcnt, tag):
                """msg partial [y, x] = sum_r praw_r o (S (x) S)(p10_r o h)"""
                msg_acc = None
                for r0 in range(0, kcnt, 8):
                    rn = min(8, kcnt - r0)
                    w0, w1 = r0 * 96, (r0 + rn) * 96
                    wp = wpool.tile([96, 8 * 96], F32, tag=f"wp{tag}")
                    nc.vector.tensor_mul(
                        wp[:, :rn * 96].rearrange("p (r x) -> p r x", r=rn),
                        p10[:, w0:w1].rearrange("p (r x) -> p r x", r=rn),
                        hy[:].unsqueeze(1).broadcast_to([96, rn, 96]))
                    # stage A (data-stationary): out_r = (Sy WP_r)^T  [x, y]
                    pt = psT.tile([96, 8 * 128], F32, tag="pt")
                    for r in range(rn):
                        nc.tensor.matmul(pt[:, r * 128:r * 128 + 96],
                                         wp[:, r * 96:(r + 1) * 96],
                                         sb["s1"][:], start=True, stop=True)
                    ts = wpool.tile([96, 8 * 96], F32, tag=f"ts{tag}")
                    nc.vector.tensor_copy(
                        ts[:, :rn * 96].rearrange("p (r y) -> p r y", r=rn),
                        pt[:].rearrange("p (r z) -> p r z", r=8)[:, :rn, 0:96])
                    # stage B (data-stationary): out_r = (Sx T_r)^T  [y, x]
                    pb = psB.tile([96, 8 * 128], F32, tag="pb")
                    for r in range(rn):
                        nc.tensor.matmul(pb[:, r * 128:r * 128 + 96],
                                         ts[:, r * 96:(r + 1) * 96],
                                         sb["s1"][:], start=True, stop=True)
                    mm = wpool.tile([96, 8 * 96], F32, tag=f"mm{tag}")
                    nc.vector.tensor_mul(
                        mm[:, :rn * 96].rearrange("p (r x) -> p r x", r=rn),
                        pb[:].rearrange("p (r z) -> p r z", r=8)[:, :rn, 0:96],
                        praw[:, w0:w1].rearrange("p (r x) -> p r x", r=rn))
                    part = wpool.tile([96, 96], F32, tag=f"part{tag}")
                    nc.vector.tensor_reduce(
                        part[:],
                        mm[:, :rn * 96].rearrange("p (r x) -> p x r", r=rn),
                        axis=mybir.AxisListType.X, op=ALU.add)
                    if msg_acc is None:
                        msg_acc = part
                    else:
                        acc2 = wpool.tile([96, 96], F32, tag=f"acc{tag}")
                        nc.vector.tensor_add(acc2[:], msg_acc[:], part[:])
                        msg_acc = acc2
                return msg_acc

            for it in range(NITER):
                if it == 0:
                    # replicated full-rank iteration: no collective needed;
                    # runs concurrently with the comm bootstrap barrier.
                    msgf = bilateral_partial(sb["pyf10"], sb["pyfraw"],
                                             KRANK, "f")
                else:
                    msg = bilateral_partial(sb["py10"], sb["pyraw"],
                                            KLOC, "s")
                    cin = dpool.tile([96, 96], F32, tag="cin")
                    cout = dpool.tile([NCORES * 96, 96], F32, tag="cout")
                    nc.sync.dma_start(cin[:], msg[:])
                    nc.gpsimd.collective_compute(
                        "AllGather", ALU.bypass, replica_groups=RG,
                        ins=[cin[:]], outs=[cout[:]])
                    gath = wpool.tile([96, NCORES * 96], F32, tag="gath")
                    nc.sync.dma_start(
                        gath[:].rearrange("p (c y) -> p c y", c=NCORES),
                        cout[:].rearrange("(c p) y -> p c y", c=NCORES))
                    msgf = wpool.tile([96, 96], F32, tag="msgf")
                    nc.vector.tensor_reduce(
                        msgf[:],
                        gath[:].rearrange("p (c y) -> p y c", c=NCORES),
                        axis=mybir.AxisListType.X, op=ALU.add)
                # gaussian term on h (off critical chain): conv_g [y, x]
                pg0 = psG.tile([96, 96], F32, tag="psg")
                nc.tensor.transpose(pg0[:], hy[:], sb["i96"][:])
                hx = wpool.tile([96, 96], F32, tag="hx")
                nc.vector.tensor_copy(hx[:], pg0[:])
                pg1 = psG.tile([96, 96], F32, tag="psg")
                nc.tensor.matmul(pg1[:], sb["g1"][:], hx[:],
                                 start=True, stop=True)          # [x,y] = G H^T
                ga = wpool.tile([96, 96], F32, tag="ga")
                nc.vector.tensor_copy(ga[:], pg1[:])
                pg2 = psG.tile([96, 96], F32, tag="psg")
                nc.tensor.transpose(pg2[:], ga[:], sb["i96"][:])  # [y,x] = H G
                gb = wpool.tile([96, 96], F32, tag="gb")
                nc.vector.tensor_copy(gb[:], pg2[:])
                pg3 = psG.tile([96, 96], F32, tag="psg")
                nc.tensor.matmul(pg3[:], sb["g1"][:], gb[:],
                                 start=True, stop=True)          # [y,x] = G H G
                # base = Cb + 3*conv_g - 13*h   (off critical chain)
                c3 = wpool.tile([96, 96], F32, tag="c3")
                nc.vector.tensor_scalar_mul(c3[:], pg3[:], 3.0)
                h13 = wpool.tile([96, 96], F32, tag="h13")
                nc.vector.tensor_scalar_mul(h13[:], hy[:], 13.0)
                b1 = wpool.tile([96, 96], F32, tag="b1")
                nc.vector.tensor_sub(b1[:], c3[:], h13[:])
                base = wpool.tile([96, 96], F32, tag="base")
                nc.vector.tensor_add(base[:], b1[:], sb["cb"][:])
                # critical chain: logit = base + msgf ; h = tanh(logit/2)
                logit = wpool.tile([96, 96], F32, tag="logit")
                nc.vector.tensor_add(logit[:], base[:], msgf[:])
                if it == NITER - 1:
                    nc.sync.dma_start(out_t[:], logit[:])
                else:
                    hy2 = cpool.tile([96, 96], F32, tag=f"hy{it}")
                    nc.scalar.activation(hy2[:], logit[:], AF.Tanh, scale=0.5)
                    hy = hy2

    nc.compile()
    return nc


def _get_nc():
    if "nc" not in _CACHE:
        _CACHE["nc"] = _build()
    return _CACHE["nc"]


# ------------------------- entry point -------------------------

def kernel(image, mask):
    global LAST_RESULTS
    import os
    from concourse.bass_utils import run_bass_kernel_spmd

    per_core, shared = _host_constants(image, mask)
    nc = _get_nc()
    in_maps = []
    for c in range(NCORES):
        m = dict(shared)
        m["py10"], m["pyraw"] = per_core[c]
        in_maps.append(m)
    trace = bool(int(os.environ.get("KERNEL_TRACE", "0")))
    kw = {}
    if trace and os.environ.get("KERNEL_TRACE_ALL"):
        kw["trace_cores"] = list(range(NCORES))
        kw["stitch_traces"] = True
    res = run_bass_kernel_spmd(nc, in_maps, core_ids=list(range(NCORES)),
                               trace=trace, **kw)
    LAST_RESULTS = res
    logit_yx = res.results[0]["logit_out"]          # [y, x]
    pred = (logit_yx < 0).astype(np.float32).reshape(1, 1, H, W)
    return pred
